# revision 14
# baseline (speedup 1.0000x reference)
"""Trainium2 Bass kernel for nn_DiscretizedMultiHeadSelfAttention.

Sharding: 8 cores = 2 batches x 4 head-groups (4 heads each), per the
data-parallel-over-batch + tensor-parallel-over-heads hint. W_o is
row-sharded, so each core emits a partial [S, D] output; the host sums the
4 partials per batch (the unshard reduction) and adds b_o.

Per core (batch b, heads hl..hl+3):
    Q = (x W_q^T + b_q)/sqrt(64); K = x W_k^T + b_k; V = (x W_v^T + b_v)/0.9
    z = Q K^T + gumbel_masked          (gumbel+mask merged on host; -inf masked)
    k* = argmax_k z                    (hard gumbel-softmax forward)
    k' = vidx[h, q, k*]                (vidx = keep ? k : S  -- dropout fold)
    y[q, :] = Vtab[h][k']              (Vtab row S = zeros)
    partial = y @ W_o[:, ch]^T

Gumbel noise / dropout keep are the deterministic jax PRNG streams (key 42)
of the reference -- input-independent constants computed on host CPU.
"""
import os
import sys
import math

sys.path.insert(0, "/opt/trn_rl_repo")

import numpy as np

B, S, D, H = 2, 2048, 1024, 16
DK = D // H
HLOC = 4
NCORES = 8
QT = S // 128
QUARTER = 512
NQMAX = S // QUARTER
EPS = 1e-20
DROP_P = 0.1

NV_ROWS = HLOC * S * S + 4096
VP_ROWS = HLOC * (S + 1)

_prog_cache = {}


def _build_program(nq, has_bq, has_bk, has_bv, debug=False,
                   v_f32r=None, stt_gp=None, zq_bufs=None, g_bufs=None,
                   split_scores=None):
    import os as _os
    if v_f32r is None: v_f32r = bool(int(_os.environ.get("V_F32R", "0")))
    if stt_gp is None: stt_gp = int(_os.environ.get("STT_GP", "0"))
    if zq_bufs is None: zq_bufs = int(_os.environ.get("ZQ_BUFS", "3"))
    no_idma = bool(int(_os.environ.get("NO_IDMA", "0")))
    if split_scores is None:
        split_scores = bool(int(_os.environ.get("SPLIT_SCORES", "1")))
    no_wo = bool(int(_os.environ.get("NO_WO", "0")))
    no_qk = bool(int(_os.environ.get("NO_QK", "0")))
    if g_bufs is None: g_bufs = int(_os.environ.get("G_BUFS", "5"))
    """Build the SPMD Bass program. nq: tuple of 16 quarter-counts per q-tile."""
    from contextlib import ExitStack
    import concourse.bass as bass
    import concourse.tile as tile
    from concourse import bacc, mybir

    f32 = mybir.dt.float32
    f32r = mybir.dt.float32r
    u16 = mybir.dt.uint16
    bf16 = mybir.dt.bfloat16
    i32 = mybir.dt.int32
    Alu = mybir.AluOpType
    AX = mybir.AxisListType

    nc = bacc.Bacc("TRN2", target_bir_lowering=False, debug=False,
                   num_devices=NCORES)

    xT = nc.dram_tensor("xT", [128, 8, S], f32, kind="ExternalInput").ap()
    wq = nc.dram_tensor("wq", [128, 8, 256], f32, kind="ExternalInput").ap()
    wk = nc.dram_tensor("wk", [128, 8, 256], f32, kind="ExternalInput").ap()
    wv = nc.dram_tensor("wv", [128, 8, 256], f32, kind="ExternalInput").ap()
    woT = nc.dram_tensor("woT", [128, 2, D], f32, kind="ExternalInput").ap()
    gmg = nc.dram_tensor("gmg", [HLOC, S, S], f32, kind="ExternalInput").ap()
    iot = nc.dram_tensor("iot", [128, S], f32, kind="ExternalInput").ap()
    rbase = nc.dram_tensor("rbase", [128, QT, HLOC], f32, kind="ExternalInput").ap()
    hbase = nc.dram_tensor("hbase", [128, HLOC], f32, kind="ExternalInput").ap()
    vidxf = nc.dram_tensor("vidxf", [NV_ROWS, 1], u16, kind="ExternalInput").ap()
    bq2 = nc.dram_tensor("bq2", [128, 2], f32, kind="ExternalInput").ap()
    bk2 = nc.dram_tensor("bk2", [128, 2], f32, kind="ExternalInput").ap()
    bvrow = nc.dram_tensor("bvrow", [1, 256], f32, kind="ExternalInput").ap()
    idn = nc.dram_tensor("idn", [128, 128], f32, kind="ExternalInput").ap()

    out_part = nc.dram_tensor("out_part", [S, D], f32, kind="ExternalOutput").ap()
    vprime = nc.dram_tensor("vprime", [VP_ROWS, DK], f32, kind="Internal").ap()
    if debug:
        d_qta = nc.dram_tensor("d_qta", [128, S], f32, kind="ExternalOutput").ap()
        d_kta = nc.dram_tensor("d_kta", [128, S], f32, kind="ExternalOutput").ap()
        d_z = nc.dram_tensor("d_z", [128, S], f32, kind="ExternalOutput").ap()
        d_ks = nc.dram_tensor("d_ks", [128, HLOC * QT], f32, kind="ExternalOutput").ap()
        d_kp = nc.dram_tensor("d_kp", [128, HLOC * QT], f32, kind="ExternalOutput").ap()
        d_y = nc.dram_tensor("d_y", [128, 256], f32, kind="ExternalOutput").ap()
        d_yta = nc.dram_tensor("d_yta", [128, S], f32, kind="ExternalOutput").ap()
        d_vp = nc.dram_tensor("d_vp", [256, DK], f32, kind="ExternalOutput").ap()

    with tile.TileContext(nc) as tc, ExitStack() as ctx:
        pp = ctx.enter_context(tc.tile_pool(name="persist", bufs=1))
        if split_scores:
            qh_a = pp.tile([128, S], bf16, tag="qha")
            qh_b = pp.tile([128, S], bf16, tag="qhb")
            ql_a = pp.tile([128, S], bf16, tag="qla")
            ql_b = pp.tile([128, S], bf16, tag="qlb")
            kh_a = pp.tile([128, S], bf16, tag="kha")
            kh_b = pp.tile([128, S], bf16, tag="khb")
            kl_a = pp.tile([128, S], bf16, tag="kla")
            kl_b = pp.tile([128, S], bf16, tag="klb")
        else:
            qt_a = pp.tile([128, S], f32, tag="qta")
            qt_b = pp.tile([128, S], f32, tag="qtb")
            kt_a = pp.tile([128, S], f32, tag="kta")
            kt_b = pp.tile([128, S], f32, tag="ktb")
        yt_a = pp.tile([128, S], f32r, tag="yta")
        yt_b = pp.tile([128, S], f32r, tag="ytb")
        iota_t = pp.tile([128, S], f32, tag="iota")
        wo_r = pp.tile([128, 2, D], f32r, tag="wor")
        rb_t = pp.tile([128, QT, HLOC], f32, tag="rb")
        hb_t = pp.tile([128, HLOC], f32, tag="hb")
        ident = pp.tile([128, 128], f32, tag="ident")

        nc.sync.dma_start(iota_t[:], iot)
        nc.sync.dma_start(rb_t[:], rbase)
        nc.sync.dma_start(hb_t[:], hbase)
        nc.sync.dma_start(ident[:], idn)

        # ---- all pools open together so phases overlap freely ----
        pps = ctx.enter_context(tc.tile_pool(name="proj_ps", bufs=2, space="PSUM"))
        zps = ctx.enter_context(tc.tile_pool(name="zps", bufs=zq_bufs, space="PSUM"))
        tps = ctx.enter_context(tc.tile_pool(name="tps", bufs=1, space="PSUM"))
        wps = ctx.enter_context(tc.tile_pool(name="wps", bufs=2, space="PSUM"))
        gpool = ctx.enter_context(tc.tile_pool(name="gpool", bufs=g_bufs))
        jpool = ctx.enter_context(tc.tile_pool(name="jpool", bufs=2))
        zpool = ctx.enter_context(tc.tile_pool(name="zpool", bufs=2))
        spool = ctx.enter_context(tc.tile_pool(name="spool", bufs=6))
        ypool = ctx.enter_context(tc.tile_pool(name="ypool", bufs=3))
        opool = ctx.enter_context(tc.tile_pool(name="opool", bufs=3))
        psb = ctx.enter_context(tc.tile_pool(name="proj_sb", bufs=3))
        xw = ctx.enter_context(tc.tile_pool(name="xw", bufs=1))
        if True:
            x_t = xw.tile([128, 8, S], f32, tag="x")
            wq_t = xw.tile([128, 8, 256], f32, tag="wqt")
            wk_t = xw.tile([128, 8, 256], f32, tag="wkt")
            wv_t = xw.tile([128, 8, 256], f32, tag="wvt")
            wo_t = xw.tile([128, 2, D], f32, tag="wot")
            nc.sync.dma_start(x_t[:], xT)
            nc.sync.dma_start(wq_t[:], wq)
            nc.sync.dma_start(wk_t[:], wk)
            nc.sync.dma_start(wv_t[:], wv)
            nc.sync.dma_start(wo_t[:], woT)
            nc.vector.tensor_copy(wo_r[:], wo_t[:])

            bq_t = bk_t = bv_t = ones_t = None
            if has_bq:
                bq_t = xw.tile([128, 2], f32, tag="bq")
                nc.sync.dma_start(bq_t[:], bq2)
            if has_bk:
                bk_t = xw.tile([128, 2], f32, tag="bk")
                nc.sync.dma_start(bk_t[:], bk2)
            if has_bv:
                bv_t = xw.tile([1, 256], f32, tag="bv")
                ones_t = xw.tile([1, 128], f32, tag="ones")
                nc.sync.dma_start(bv_t[:], bvrow)
                nc.vector.memset(ones_t[:], 1.0)

            if split_scores:
                proj_list = () if no_qk else (
                    (wq_t, ((qh_a, ql_a), (qh_b, ql_b)), bq_t),
                    (wk_t, ((kh_a, kl_a), (kh_b, kl_b)), bk_t),
                )
                for (w_t, dsts, b_t) in proj_list:
                    for dblk in range(2):
                        hi, lo = dsts[dblk]
                        for sblk in range(4):
                            ps = pps.tile([128, QUARTER], f32, space="PSUM", tag="pps")
                            for i in range(8):
                                nc.tensor.matmul(
                                    ps[:],
                                    w_t[:, i, dblk * 128:(dblk + 1) * 128],
                                    x_t[:, i, sblk * QUARTER:(sblk + 1) * QUARTER],
                                    start=(i == 0), stop=(i == 7))
                            ssl = slice(sblk * QUARTER, (sblk + 1) * QUARTER)
                            if b_t is not None:
                                nc.scalar.add(hi[:, ssl], ps[:], b_t[:, dblk:dblk + 1])
                                nc.vector.scalar_tensor_tensor(
                                    out=lo[:, ssl], in0=ps[:],
                                    scalar=b_t[:, dblk:dblk + 1], in1=hi[:, ssl],
                                    op0=Alu.add, op1=Alu.subtract)
                            else:
                                nc.scalar.copy(hi[:, ssl], ps[:])
                                nc.vector.tensor_tensor(
                                    out=lo[:, ssl], in0=ps[:], in1=hi[:, ssl],
                                    op=Alu.subtract)
            else:
              for (w_t, dsts, b_t) in (() if no_qk else (
                (wq_t, (qt_a, qt_b), bq_t),
                (wk_t, (kt_a, kt_b), bk_t),
              )):
                for dblk in range(2):
                    for sblk in range(4):
                        ps = pps.tile([128, QUARTER], f32, space="PSUM", tag="pps")
                        for i in range(8):
                            nc.tensor.matmul(
                                ps[:],
                                w_t[:, i, dblk * 128:(dblk + 1) * 128],
                                x_t[:, i, sblk * QUARTER:(sblk + 1) * QUARTER],
                                start=(i == 0), stop=(i == 7))
                        sl = dsts[dblk][:, sblk * QUARTER:(sblk + 1) * QUARTER]
                        if b_t is not None:
                            nc.scalar.add(sl, ps[:], b_t[:, dblk:dblk + 1])
                        else:
                            nc.scalar.copy(sl, ps[:])

            # V: out[s 128, c 256]; DMA rows into vprime (+ zero row per head)
            zrow = psb.tile([1, DK], f32, tag="zrow")
            nc.vector.memset(zrow[:], 0.0)
            for h in range(HLOC):
                nc.sync.dma_start(
                    vprime[h * (S + 1) + S:h * (S + 1) + S + 1, :], zrow[:])
            if v_f32r:
                x_r = xw.tile([128, 8, S], f32r, tag="xr")
                wv_r = xw.tile([128, 8, 256], f32r, tag="wvr")
                nc.vector.tensor_copy(x_r[:], x_t[:])
                nc.vector.tensor_copy(wv_r[:], wv_t[:])
            for st in range(QT):
                ps = pps.tile([128, 256], f32, space="PSUM", tag="pps")
                for i in range(8):
                    if v_f32r:
                        nc.tensor.matmul(
                            ps[:],
                            x_r[:, i, st * 128:(st + 1) * 128],
                            wv_r[:, i, :],
                            start=(i == 0), stop=(i == 7 and not has_bv))
                        continue
                    nc.tensor.matmul(
                        ps[:],
                        x_t[:, i, st * 128:(st + 1) * 128],
                        wv_t[:, i, :],
                        start=(i == 0), stop=(i == 7 and not has_bv))
                if has_bv:
                    nc.tensor.matmul(ps[:], ones_t[:], bv_t[:],
                                     start=False, stop=True)
                v_sb = psb.tile([128, 256], f32, tag="vsb")
                nc.scalar.copy(v_sb[:], ps[:])
                for h in range(HLOC):
                    nc.sync.dma_start(
                        vprime[h * (S + 1) + st * 128:
                               h * (S + 1) + (st + 1) * 128, :],
                        v_sb[:, h * DK:(h + 1) * DK])

        if debug:
            nc.sync.dma_start(d_qta, qt_a[:])
            nc.sync.dma_start(d_kta, kt_a[:])
            with tc.tile_pool(name="dvp", bufs=1) as dvp:
                vv = dvp.tile([128, DK], f32, tag="vv")
                nc.sync.dma_start(vv[:], vprime[0:128, :])
                nc.sync.dma_start(d_vp[0:128, :], vv[:])
                vv2 = dvp.tile([128, DK], f32, tag="vv2")
                nc.sync.dma_start(vv2[:], vprime[(S + 1) * 1:(S + 1) * 1 + 128, :])
                nc.sync.dma_start(d_vp[128:256, :], vv2[:])

        # ---------------- phase B/C/D ----------------
        if True:
            for qt in range(QT):
                nqt = nq[qt]
                ks4 = spool.tile([128, HLOC], f32, tag="ks4")
                for h in range(HLOC):
                    if split_scores:
                        QH = qh_a if h < 2 else qh_b
                        QL = ql_a if h < 2 else ql_b
                        KH = kh_a if h < 2 else kh_b
                        KL = kl_a if h < 2 else kl_b
                    else:
                        grp = qt_a if h < 2 else qt_b
                        krp = kt_a if h < 2 else kt_b
                    p0 = (h % 2) * 64
                    z_t = zpool.tile([128, S], f32, tag="z")
                    m4 = spool.tile([128, NQMAX], f32, tag="m4")
                    for c in range(nqt):
                        g_t = gpool.tile([128, QUARTER], f32, tag="g")
                        nc.sync.dma_start(
                            g_t[:],
                            gmg[h, qt * 128:(qt + 1) * 128,
                                c * QUARTER:(c + 1) * QUARTER])
                        ps = zps.tile([128, QUARTER], f32, space="PSUM", tag="zq")
                        if split_scores:
                            qsl = slice(qt * 128, (qt + 1) * 128)
                            csl = slice(c * QUARTER, (c + 1) * QUARTER)
                            rsl = slice(p0, p0 + 64)
                            nc.tensor.matmul(ps[:], QH[rsl, qsl], KH[rsl, csl],
                                             start=True, stop=False)
                            nc.tensor.matmul(ps[:], QL[rsl, qsl], KH[rsl, csl],
                                             start=False, stop=False)
                            nc.tensor.matmul(ps[:], QH[rsl, qsl], KL[rsl, csl],
                                             start=False, stop=True)
                        else:
                            nc.tensor.matmul(
                                ps[:],
                                grp[p0:p0 + 64, qt * 128:(qt + 1) * 128],
                                krp[p0:p0 + 64, c * QUARTER:(c + 1) * QUARTER],
                                start=True, stop=True)
                        zsl = z_t[:, c * QUARTER:(c + 1) * QUARTER]
                        nc.vector.tensor_add(zsl, ps[:], g_t[:])
                        nc.vector.tensor_reduce(
                            out=m4[:, c:c + 1], in_=zsl, axis=AX.X, op=Alu.max)
                    mg = spool.tile([128, 1], f32, tag="mg")
                    if nqt > 1:
                        nc.vector.tensor_reduce(
                            out=mg[:], in_=m4[:, 0:nqt], axis=AX.X, op=Alu.max)
                    else:
                        nc.vector.tensor_copy(mg[:], m4[:, 0:1])
                    kq4 = spool.tile([128, NQMAX], f32, tag="kq4")
                    eng = nc.gpsimd if (stt_gp and ((qt * HLOC + h) % stt_gp == 0)) else nc.vector
                    for c in range(nqt):
                        junk = jpool.tile([128, QUARTER], f32, tag="junk")
                        eng.scalar_tensor_tensor(
                            out=junk[:],
                            in0=z_t[:, c * QUARTER:(c + 1) * QUARTER],
                            scalar=mg[:, 0:1],
                            in1=iota_t[:, c * QUARTER:(c + 1) * QUARTER],
                            op0=Alu.is_equal, op1=Alu.mult,
                            accum_out=kq4[:, c:c + 1])
                    if nqt > 1:
                        nc.vector.tensor_reduce(
                            out=ks4[:, h:h + 1], in_=kq4[:, 0:nqt],
                            axis=AX.X, op=Alu.add)
                    else:
                        nc.vector.tensor_copy(ks4[:, h:h + 1], kq4[:, 0:1])
                    if debug and qt == QT - 1 and h == 0:
                        nc.sync.dma_start(d_z, z_t[:])

                # clamp garbage (fully-masked rows) into table range
                ks4c = spool.tile([128, HLOC], f32, tag="ks4c")
                nc.gpsimd.tensor_scalar_min(ks4c[:], ks4[:], float(S))
                ko4 = spool.tile([128, HLOC], f32, tag="ko4")
                nc.gpsimd.tensor_add(ko4[:], ks4c[:], rb_t[:, qt, :])
                ko4i = spool.tile([128, HLOC], i32, tag="ko4i")
                nc.gpsimd.tensor_copy(ko4i[:], ko4[:])
                kp4 = spool.tile([128, HLOC], u16, tag="kp4")
                if no_idma:
                    nc.vector.memset(kp4[:], 0)
                else:
                    for h in range(HLOC):
                        nc.gpsimd.indirect_dma_start(
                            out=kp4[:, h:h + 1], out_offset=None,
                            in_=vidxf,
                            in_offset=bass.IndirectOffsetOnAxis(
                                ap=ko4i[:, h:h + 1], axis=0))
                kp4f = spool.tile([128, HLOC], f32, tag="kp4f")
                nc.gpsimd.tensor_copy(kp4f[:], kp4[:])
                vo4 = spool.tile([128, HLOC], f32, tag="vo4")
                nc.gpsimd.tensor_add(vo4[:], kp4f[:], hb_t[:])
                vo4i = spool.tile([128, HLOC], i32, tag="vo4i")
                nc.gpsimd.tensor_copy(vo4i[:], vo4[:])
                y_t = ypool.tile([128, 256], f32, tag="y")
                if no_idma:
                    nc.vector.memset(y_t[:], 0.0)
                else:
                    for h in range(HLOC):
                        nc.gpsimd.indirect_dma_start(
                            out=y_t[:, h * DK:(h + 1) * DK], out_offset=None,
                            in_=vprime,
                            in_offset=bass.IndirectOffsetOnAxis(
                                ap=vo4i[:, h:h + 1], axis=0))
                if debug:
                    nc.sync.dma_start(d_ks[:, qt * HLOC:(qt + 1) * HLOC], ks4[:])
                    nc.sync.dma_start(d_kp[:, qt * HLOC:(qt + 1) * HLOC], kp4f[:])
                    if qt == 0:
                        nc.sync.dma_start(d_y, y_t[:])

                for cb, dst in ((0, yt_a), (1, yt_b)):
                    tp = tps.tile([128, 128], f32, space="PSUM", tag="tp")
                    nc.tensor.transpose(tp[:], y_t[:, cb * 128:(cb + 1) * 128],
                                        ident[:])
                    nc.scalar.copy(dst[:, qt * 128:(qt + 1) * 128], tp[:])

            if debug:
                nc.sync.dma_start(d_yta, yt_a[:].bitcast(f32))
            for st in range(QT if not no_wo else 0):
                for ob in range(2):
                    ps = wps.tile([128, QUARTER], f32, space="PSUM", tag="wops")
                    nc.tensor.matmul(
                        ps[:], yt_a[:, st * 128:(st + 1) * 128],
                        wo_r[:, 0, ob * QUARTER:(ob + 1) * QUARTER],
                        start=True, stop=False)
                    nc.tensor.matmul(
                        ps[:], yt_b[:, st * 128:(st + 1) * 128],
                        wo_r[:, 1, ob * QUARTER:(ob + 1) * QUARTER],
                        start=False, stop=True)
                    o_sb = opool.tile([128, QUARTER], f32, tag="osb")
                    nc.scalar.copy(o_sb[:], ps[:])
                    nc.sync.dma_start(
                        out_part[st * 128:(st + 1) * 128,
                                 ob * QUARTER:(ob + 1) * QUARTER],
                        o_sb[:])

    nc.compile()
    return nc


def _get_program(nq, has_bq, has_bk, has_bv):
    key = (tuple(nq), has_bq, has_bk, has_bv)
    if key not in _prog_cache:
        _prog_cache[key] = _build_program(nq, has_bq, has_bk, has_bv)
    return _prog_cache[key]


def _gumbel_keep():
    """Reproduce the reference's PRNG streams on host CPU."""
    import jax
    cpu = jax.devices("cpu")[0]
    with jax.default_device(cpu):
        import jax.numpy as jnp
        nkey = jax.random.key(42)
        k_g, k_d = jax.random.split(nkey)
        shape = (B, H, S, S)
        u = jax.random.uniform(k_g, shape, dtype=jnp.float32)
        gumbel = -jnp.log(-jnp.log(u + EPS) + EPS)
        keep = jax.random.bernoulli(k_d, 1.0 - DROP_P, shape)
        return np.asarray(gumbel), np.asarray(keep)


_rand_cache = {}


def _core_inputs(x, W_q, W_k, W_v, W_o, b_q, b_k, b_v, gm, vidx):
    p_idx = np.arange(128, dtype=np.float32)
    iota = np.broadcast_to(np.arange(S, dtype=np.float32), (128, S)).copy()
    idn = np.eye(128, dtype=np.float32)
    qt_idx = np.arange(QT, dtype=np.float32)
    h_idx = np.arange(HLOC, dtype=np.float32)
    rbase = ((h_idx[None, None, :] * S + qt_idx[None, :, None] * 128
              + p_idx[:, None, None]) * S).astype(np.float32)
    hbase = np.broadcast_to(h_idx[None, :] * (S + 1),
                            (128, HLOC)).astype(np.float32).copy()

    in_maps = []
    for core in range(NCORES):
        b = core // 4
        hg = core % 4
        hsl = slice(hg * HLOC, (hg + 1) * HLOC)

        xT_re = np.ascontiguousarray(x[b].T.reshape(8, 128, S).transpose(1, 0, 2))

        def w_re(W, scale):
            W_loc = W[hg * 256:(hg + 1) * 256, :] * scale
            return np.ascontiguousarray(
                W_loc.T.reshape(8, 128, 256).transpose(1, 0, 2)).astype(np.float32)

        wq_re = w_re(W_q, 1.0 / math.sqrt(DK))
        wk_re = w_re(W_k, 1.0)
        wv_re = w_re(W_v, 1.0 / (1.0 - DROP_P))

        Wo_loc = W_o[:, hg * 256:(hg + 1) * 256]
        woT_re = np.ascontiguousarray(
            Wo_loc.T.reshape(2, 128, D).transpose(1, 0, 2)).astype(np.float32)

        gmg_c = np.ascontiguousarray(gm[b, hsl])
        vidx_c = vidx[b, hsl].reshape(-1, 1)
        vidxf = np.ascontiguousarray(np.concatenate(
            [vidx_c, np.zeros((4096, 1), np.uint16)], axis=0))

        bq_re = np.ascontiguousarray(
            (b_q[hg * 256:(hg + 1) * 256] / math.sqrt(DK)).reshape(2, 128).T
        ).astype(np.float32)
        bk_re = np.ascontiguousarray(
            b_k[hg * 256:(hg + 1) * 256].reshape(2, 128).T).astype(np.float32)
        bv_re = (b_v[hg * 256:(hg + 1) * 256] / (1.0 - DROP_P)
                 ).reshape(1, 256).astype(np.float32)

        in_maps.append(dict(
            xT=xT_re, wq=wq_re, wk=wk_re, wv=wv_re, woT=woT_re,
            gmg=gmg_c, iot=iota, rbase=rbase, hbase=hbase, vidxf=vidxf,
            bq2=bq_re, bk2=bk_re, bvrow=bv_re, idn=idn,
        ))
    return in_maps


def kernel(**inputs):
    import concourse.bass_utils as bass_utils

    x = np.asarray(inputs["x"], dtype=np.float32)
    W_q = np.asarray(inputs["W_q"], dtype=np.float32)
    W_k = np.asarray(inputs["W_k"], dtype=np.float32)
    W_v = np.asarray(inputs["W_v"], dtype=np.float32)
    W_o = np.asarray(inputs["W_o"], dtype=np.float32)
    b_q = np.asarray(inputs["b_q"], dtype=np.float32)
    b_k = np.asarray(inputs["b_k"], dtype=np.float32)
    b_v = np.asarray(inputs["b_v"], dtype=np.float32)
    b_o = np.asarray(inputs["b_o"], dtype=np.float32)
    mask = np.asarray(inputs["mask"])
    hard = int(np.asarray(inputs["hard"]))

    if hard != 1:
        raise NotImplementedError("only hard=1 supported")

    if "gk" not in _rand_cache:
        _rand_cache["gk"] = _gumbel_keep()
    gumbel, keep = _rand_cache["gk"]

    gm = np.where((mask[:, None, :, :] == 0), np.float32(-np.inf), gumbel)
    ar = np.arange(S, dtype=np.uint16)
    vidx = np.where(keep, ar[None, None, None, :], np.uint16(S))

    nq = []
    anymask = (mask != 0)
    for qt in range(QT):
        sub = anymask[:, qt * 128:(qt + 1) * 128, :]
        cols = np.nonzero(sub.any(axis=(0, 1)))[0]
        lastk = int(cols.max()) if cols.size else 0
        nq.append(min(NQMAX, max(1, -(-(lastk + 1) // QUARTER))))
    nq = tuple(nq)

    has_bq = bool(np.any(b_q != 0))
    has_bk = bool(np.any(b_k != 0))
    has_bv = bool(np.any(b_v != 0))

    nc = _get_program(nq, has_bq, has_bk, has_bv)
    in_maps = _core_inputs(x, W_q, W_k, W_v, W_o, b_q, b_k, b_v, gm, vidx)
    global _last_in_maps
    _last_in_maps = in_maps

    res = bass_utils.run_bass_kernel_spmd(
        nc, in_maps, core_ids=list(range(NCORES)))

    out = np.zeros((B, S, D), dtype=np.float32)
    for core in range(NCORES):
        out[core // 4] += res.results[core]["out_part"]
    out += b_o[None, None, :]
    return out


# revision 17
# speedup vs baseline: 1.3973x; 1.3973x over previous
"""Trainium2 Bass kernel for nn_DiscretizedMultiHeadSelfAttention.

Sharding: 8 cores = 2 batches x 4 head-groups (4 heads each), per the
data-parallel-over-batch + tensor-parallel-over-heads hint. W_o is
row-sharded, so each core emits a partial [S, D] output; the host sums the
4 partials per batch (the unshard reduction) and adds b_o.

Per core (batch b, heads hl..hl+3):
    Q = (x W_q^T + b_q)/sqrt(64); K = x W_k^T + b_k; V = (x W_v^T + b_v)/0.9
    z = Q K^T + gumbel_masked          (gumbel+mask merged on host; -inf masked)
    k* = argmax_k z                    (hard gumbel-softmax forward)
    k' = vidx[h, q, k*]                (vidx = keep ? k : S  -- dropout fold)
    y[q, :] = Vtab[h][k']              (Vtab row S = zeros)
    partial = y @ W_o[:, ch]^T

Gumbel noise / dropout keep are the deterministic jax PRNG streams (key 42)
of the reference -- input-independent constants computed on host CPU.
"""
import os
import sys
import math

sys.path.insert(0, "/opt/trn_rl_repo")

import numpy as np

B, S, D, H = 2, 2048, 1024, 16
DK = D // H
HLOC = 4
NCORES = 8
QT = S // 128
QUARTER = 512
NQMAX = S // QUARTER
EPS = 1e-20
DROP_P = 0.1

NV_ROWS = HLOC * S * S + 4096
VP_ROWS = HLOC * (S + 1)

_prog_cache = {}


def _build_program(nq, has_bq, has_bk, has_bv, debug=False,
                   v_f32r=None, stt_gp=None, zq_bufs=None, g_bufs=None,
                   split_scores=None):
    import os as _os
    if v_f32r is None: v_f32r = bool(int(_os.environ.get("V_F32R", "0")))
    if stt_gp is None: stt_gp = int(_os.environ.get("STT_GP", "0"))
    if zq_bufs is None: zq_bufs = int(_os.environ.get("ZQ_BUFS", "3"))
    no_idma = bool(int(_os.environ.get("NO_IDMA", "0")))
    if split_scores is None:
        split_scores = bool(int(_os.environ.get("SPLIT_SCORES", "1")))
    no_wo = bool(int(_os.environ.get("NO_WO", "0")))
    no_qk = bool(int(_os.environ.get("NO_QK", "0")))
    if g_bufs is None: g_bufs = int(_os.environ.get("G_BUFS", "5"))
    """Build the SPMD Bass program. nq: tuple of 16 quarter-counts per q-tile."""
    from contextlib import ExitStack
    import concourse.bass as bass
    import concourse.tile as tile
    from concourse import bacc, mybir

    f32 = mybir.dt.float32
    f32r = mybir.dt.float32r
    u16 = mybir.dt.uint16
    bf16 = mybir.dt.bfloat16
    i32 = mybir.dt.int32
    Alu = mybir.AluOpType
    AX = mybir.AxisListType

    nc = bacc.Bacc("TRN2", target_bir_lowering=False, debug=False,
                   num_devices=NCORES)

    xT = nc.dram_tensor("xT", [128, 8, S], f32, kind="ExternalInput").ap()
    wq = nc.dram_tensor("wq", [128, 8, 256], f32, kind="ExternalInput").ap()
    wk = nc.dram_tensor("wk", [128, 8, 256], f32, kind="ExternalInput").ap()
    wv = nc.dram_tensor("wv", [128, 8, 256], f32, kind="ExternalInput").ap()
    woT = nc.dram_tensor("woT", [128, 2, D], f32, kind="ExternalInput").ap()
    gmg = nc.dram_tensor("gmg", [HLOC, S, S], f32, kind="ExternalInput").ap()
    iot = nc.dram_tensor("iot", [128, S], f32, kind="ExternalInput").ap()
    rbase = nc.dram_tensor("rbase", [128, QT, HLOC], f32, kind="ExternalInput").ap()
    hbase = nc.dram_tensor("hbase", [128, HLOC], f32, kind="ExternalInput").ap()
    vidxf = nc.dram_tensor("vidxf", [NV_ROWS, 1], u16, kind="ExternalInput").ap()
    bq2 = nc.dram_tensor("bq2", [128, 2], f32, kind="ExternalInput").ap()
    bk2 = nc.dram_tensor("bk2", [128, 2], f32, kind="ExternalInput").ap()
    bvrow = nc.dram_tensor("bvrow", [1, 256], f32, kind="ExternalInput").ap()
    idn = nc.dram_tensor("idn", [128, 128], f32, kind="ExternalInput").ap()

    out_part = nc.dram_tensor("out_part", [S, D], f32, kind="ExternalOutput").ap()
    vprime = nc.dram_tensor("vprime", [VP_ROWS, DK], f32, kind="Internal").ap()
    if debug:
        d_qta = nc.dram_tensor("d_qta", [128, S], f32, kind="ExternalOutput").ap()
        d_kta = nc.dram_tensor("d_kta", [128, S], f32, kind="ExternalOutput").ap()
        d_z = nc.dram_tensor("d_z", [128, S], f32, kind="ExternalOutput").ap()
        d_ks = nc.dram_tensor("d_ks", [128, HLOC * QT], f32, kind="ExternalOutput").ap()
        d_kp = nc.dram_tensor("d_kp", [128, HLOC * QT], f32, kind="ExternalOutput").ap()
        d_y = nc.dram_tensor("d_y", [128, 256], f32, kind="ExternalOutput").ap()
        d_yta = nc.dram_tensor("d_yta", [128, S], f32, kind="ExternalOutput").ap()
        d_vp = nc.dram_tensor("d_vp", [256, DK], f32, kind="ExternalOutput").ap()

    with tile.TileContext(nc) as tc, ExitStack() as ctx:
        pp = ctx.enter_context(tc.tile_pool(name="persist", bufs=1))
        if split_scores:
            def sblk_tiles(tag):
                out = []
                for s in range(4):
                    t = pp.tile([128, QUARTER], bf16, tag=f"{tag}{s}",
                                name=f"{tag}{s}")
                    out.append(t)
                return out
            qh_a, qh_b = sblk_tiles("qha"), sblk_tiles("qhb")
            ql_a, ql_b = sblk_tiles("qla"), sblk_tiles("qlb")
            kh_a, kh_b = sblk_tiles("kha"), sblk_tiles("khb")
            kl_a, kl_b = sblk_tiles("kla"), sblk_tiles("klb")
        else:
            qt_a = pp.tile([128, S], f32, tag="qta")
            qt_b = pp.tile([128, S], f32, tag="qtb")
            kt_a = pp.tile([128, S], f32, tag="kta")
            kt_b = pp.tile([128, S], f32, tag="ktb")
        yt_a = pp.tile([128, S], f32r, tag="yta")
        yt_b = pp.tile([128, S], f32r, tag="ytb")
        iota_t = pp.tile([128, S], f32, tag="iota")
        wo_r = pp.tile([128, 2, D], f32r, tag="wor")
        rb_t = pp.tile([128, QT, HLOC], f32, tag="rb")
        hb_t = pp.tile([128, HLOC], f32, tag="hb")
        ident = pp.tile([128, 128], f32, tag="ident")

        nc.sync.dma_start(iota_t[:], iot)
        nc.sync.dma_start(rb_t[:], rbase)
        nc.sync.dma_start(hb_t[:], hbase)
        nc.sync.dma_start(ident[:], idn)

        # ---- all pools open together so phases overlap freely ----
        pps = ctx.enter_context(tc.tile_pool(name="proj_ps", bufs=2, space="PSUM"))
        zps = ctx.enter_context(tc.tile_pool(name="zps", bufs=zq_bufs, space="PSUM"))
        tps = ctx.enter_context(tc.tile_pool(name="tps", bufs=1, space="PSUM"))
        wps = ctx.enter_context(tc.tile_pool(name="wps", bufs=2, space="PSUM"))
        gpool = ctx.enter_context(tc.tile_pool(name="gpool", bufs=g_bufs))
        jpool = ctx.enter_context(tc.tile_pool(name="jpool", bufs=2))
        zpool = ctx.enter_context(tc.tile_pool(name="zpool", bufs=2))
        spool = ctx.enter_context(tc.tile_pool(name="spool", bufs=6))
        ypool = ctx.enter_context(tc.tile_pool(name="ypool", bufs=3))
        opool = ctx.enter_context(tc.tile_pool(name="opool", bufs=3))
        psb = ctx.enter_context(tc.tile_pool(name="proj_sb", bufs=3))
        xw = ctx.enter_context(tc.tile_pool(name="xw", bufs=1))
        if True:
            x_t = xw.tile([128, 8, S], f32, tag="x")
            wq_t = xw.tile([128, 8, 256], f32, tag="wqt")
            wk_t = xw.tile([128, 8, 256], f32, tag="wkt")
            wv_t = xw.tile([128, 8, 256], f32, tag="wvt")
            wo_t = xw.tile([128, 2, D], f32, tag="wot")
            for i in range(8):
                nc.sync.dma_start(x_t[:, i, :], xT[:, i, :])
            nc.sync.dma_start(wq_t[:], wq)
            nc.sync.dma_start(wk_t[:], wk)
            nc.sync.dma_start(wv_t[:], wv)

            bq_t = bk_t = bv_t = ones_t = None
            if has_bq:
                bq_t = xw.tile([128, 2], f32, tag="bq")
                nc.sync.dma_start(bq_t[:], bq2)
            if has_bk:
                bk_t = xw.tile([128, 2], f32, tag="bk")
                nc.sync.dma_start(bk_t[:], bk2)
            if has_bv:
                bv_t = xw.tile([1, 256], f32, tag="bv")
                ones_t = xw.tile([1, 128], f32, tag="ones")
                nc.sync.dma_start(bv_t[:], bvrow)
                nc.vector.memset(ones_t[:], 1.0)

            if split_scores:
                proj_list = () if no_qk else (
                    (wq_t, ((qh_a, ql_a), (qh_b, ql_b)), bq_t),
                    (wk_t, ((kh_a, kl_a), (kh_b, kl_b)), bk_t),
                )
                for dblk in range(2):
                    for sblk in range(4):
                        for (w_t, dsts, b_t) in proj_list:
                            hi, lo = dsts[dblk]
                            ps = pps.tile([128, QUARTER], f32, space="PSUM", tag="pps")
                            for i in range(8):
                                nc.tensor.matmul(
                                    ps[:],
                                    w_t[:, i, dblk * 128:(dblk + 1) * 128],
                                    x_t[:, i, sblk * QUARTER:(sblk + 1) * QUARTER],
                                    start=(i == 0), stop=(i == 7))
                            if b_t is not None:
                                nc.scalar.add(hi[sblk][:], ps[:], b_t[:, dblk:dblk + 1])
                                nc.vector.scalar_tensor_tensor(
                                    out=lo[sblk][:], in0=ps[:],
                                    scalar=b_t[:, dblk:dblk + 1], in1=hi[sblk][:],
                                    op0=Alu.add, op1=Alu.subtract)
                            else:
                                nc.scalar.copy(hi[sblk][:], ps[:])
                                nc.vector.tensor_tensor(
                                    out=lo[sblk][:], in0=ps[:], in1=hi[sblk][:],
                                    op=Alu.subtract)
            else:
              for (w_t, dsts, b_t) in (() if no_qk else (
                (wq_t, (qt_a, qt_b), bq_t),
                (wk_t, (kt_a, kt_b), bk_t),
              )):
                for dblk in range(2):
                    for sblk in range(4):
                        ps = pps.tile([128, QUARTER], f32, space="PSUM", tag="pps")
                        for i in range(8):
                            nc.tensor.matmul(
                                ps[:],
                                w_t[:, i, dblk * 128:(dblk + 1) * 128],
                                x_t[:, i, sblk * QUARTER:(sblk + 1) * QUARTER],
                                start=(i == 0), stop=(i == 7))
                        sl = dsts[dblk][:, sblk * QUARTER:(sblk + 1) * QUARTER]
                        if b_t is not None:
                            nc.scalar.add(sl, ps[:], b_t[:, dblk:dblk + 1])
                        else:
                            nc.scalar.copy(sl, ps[:])

            # V: out[s 128, c 256]; DMA rows into vprime (+ zero row per head)
            zrow = psb.tile([1, DK], f32, tag="zrow")
            nc.vector.memset(zrow[:], 0.0)
            for h in range(HLOC):
                nc.sync.dma_start(
                    vprime[h * (S + 1) + S:h * (S + 1) + S + 1, :], zrow[:])
            if v_f32r:
                x_r = xw.tile([128, 8, S], f32r, tag="xr")
                wv_r = xw.tile([128, 8, 256], f32r, tag="wvr")
                nc.vector.tensor_copy(x_r[:], x_t[:])
                nc.vector.tensor_copy(wv_r[:], wv_t[:])
            for st in range(QT):
                ps = pps.tile([128, 256], f32, space="PSUM", tag="pps")
                for i in range(8):
                    if v_f32r:
                        nc.tensor.matmul(
                            ps[:],
                            x_r[:, i, st * 128:(st + 1) * 128],
                            wv_r[:, i, :],
                            start=(i == 0), stop=(i == 7 and not has_bv))
                        continue
                    nc.tensor.matmul(
                        ps[:],
                        x_t[:, i, st * 128:(st + 1) * 128],
                        wv_t[:, i, :],
                        start=(i == 0), stop=(i == 7 and not has_bv))
                if has_bv:
                    nc.tensor.matmul(ps[:], ones_t[:], bv_t[:],
                                     start=False, stop=True)
                v_sb = psb.tile([128, 256], f32, tag="vsb")
                nc.scalar.copy(v_sb[:], ps[:])
                for h in range(HLOC):
                    nc.sync.dma_start(
                        vprime[h * (S + 1) + st * 128:
                               h * (S + 1) + (st + 1) * 128, :],
                        v_sb[:, h * DK:(h + 1) * DK])

        if debug:
            nc.sync.dma_start(d_qta, qt_a[:])
            nc.sync.dma_start(d_kta, kt_a[:])
            with tc.tile_pool(name="dvp", bufs=1) as dvp:
                vv = dvp.tile([128, DK], f32, tag="vv")
                nc.sync.dma_start(vv[:], vprime[0:128, :])
                nc.sync.dma_start(d_vp[0:128, :], vv[:])
                vv2 = dvp.tile([128, DK], f32, tag="vv2")
                nc.sync.dma_start(vv2[:], vprime[(S + 1) * 1:(S + 1) * 1 + 128, :])
                nc.sync.dma_start(d_vp[128:256, :], vv2[:])

        # ---------------- phase B/C/D ----------------
        if True:
            for qt in range(QT):
                nqt = nq[qt]
                ks4 = spool.tile([128, HLOC], f32, tag="ks4")
                for h in range(HLOC):
                    if split_scores:
                        QH = qh_a if h < 2 else qh_b
                        QL = ql_a if h < 2 else ql_b
                        KH = kh_a if h < 2 else kh_b
                        KL = kl_a if h < 2 else kl_b
                    else:
                        grp = qt_a if h < 2 else qt_b
                        krp = kt_a if h < 2 else kt_b
                    p0 = (h % 2) * 64
                    z_t = zpool.tile([128, S], f32, tag="z")
                    m4 = spool.tile([128, NQMAX], f32, tag="m4")
                    for c in range(nqt):
                        g_t = gpool.tile([128, QUARTER], f32, tag="g")
                        nc.sync.dma_start(
                            g_t[:],
                            gmg[h, qt * 128:(qt + 1) * 128,
                                c * QUARTER:(c + 1) * QUARTER])
                        ps = zps.tile([128, QUARTER], f32, space="PSUM", tag="zq")
                        if split_scores:
                            qs, ql_ = qt // 4, (qt % 4) * 128
                            qh_sl = QH[qs][p0:p0 + 64, ql_:ql_ + 128]
                            qlo_sl = QL[qs][p0:p0 + 64, ql_:ql_ + 128]
                            kh_sl = KH[c][p0:p0 + 64, :]
                            kl_sl = KL[c][p0:p0 + 64, :]
                            nc.tensor.matmul(ps[:], qh_sl, kh_sl,
                                             start=True, stop=False)
                            nc.tensor.matmul(ps[:], qlo_sl, kh_sl,
                                             start=False, stop=False)
                            nc.tensor.matmul(ps[:], qh_sl, kl_sl,
                                             start=False, stop=True)
                        else:
                            nc.tensor.matmul(
                                ps[:],
                                grp[p0:p0 + 64, qt * 128:(qt + 1) * 128],
                                krp[p0:p0 + 64, c * QUARTER:(c + 1) * QUARTER],
                                start=True, stop=True)
                        zsl = z_t[:, c * QUARTER:(c + 1) * QUARTER]
                        nc.vector.tensor_add(zsl, ps[:], g_t[:])
                        nc.vector.tensor_reduce(
                            out=m4[:, c:c + 1], in_=zsl, axis=AX.X, op=Alu.max)
                    mg = spool.tile([128, 1], f32, tag="mg")
                    if nqt > 1:
                        nc.vector.tensor_reduce(
                            out=mg[:], in_=m4[:, 0:nqt], axis=AX.X, op=Alu.max)
                    else:
                        nc.vector.tensor_copy(mg[:], m4[:, 0:1])
                    kq4 = spool.tile([128, NQMAX], f32, tag="kq4")
                    eng = nc.gpsimd if (stt_gp and ((qt * HLOC + h) % stt_gp == 0)) else nc.vector
                    for c in range(nqt):
                        junk = jpool.tile([128, QUARTER], f32, tag="junk")
                        eng.scalar_tensor_tensor(
                            out=junk[:],
                            in0=z_t[:, c * QUARTER:(c + 1) * QUARTER],
                            scalar=mg[:, 0:1],
                            in1=iota_t[:, c * QUARTER:(c + 1) * QUARTER],
                            op0=Alu.is_equal, op1=Alu.mult,
                            accum_out=kq4[:, c:c + 1])
                    if nqt > 1:
                        nc.vector.tensor_reduce(
                            out=ks4[:, h:h + 1], in_=kq4[:, 0:nqt],
                            axis=AX.X, op=Alu.add)
                    else:
                        nc.vector.tensor_copy(ks4[:, h:h + 1], kq4[:, 0:1])
                    if debug and qt == QT - 1 and h == 0:
                        nc.sync.dma_start(d_z, z_t[:])

                # clamp garbage (fully-masked rows) into table range
                ks4c = spool.tile([128, HLOC], f32, tag="ks4c")
                nc.gpsimd.tensor_scalar_min(ks4c[:], ks4[:], float(S))
                ko4 = spool.tile([128, HLOC], f32, tag="ko4")
                nc.gpsimd.tensor_add(ko4[:], ks4c[:], rb_t[:, qt, :])
                ko4i = spool.tile([128, HLOC], i32, tag="ko4i")
                nc.gpsimd.tensor_copy(ko4i[:], ko4[:])
                kp4 = spool.tile([128, HLOC], u16, tag="kp4")
                if no_idma:
                    nc.vector.memset(kp4[:], 0)
                else:
                    for h in range(HLOC):
                        nc.gpsimd.indirect_dma_start(
                            out=kp4[:, h:h + 1], out_offset=None,
                            in_=vidxf,
                            in_offset=bass.IndirectOffsetOnAxis(
                                ap=ko4i[:, h:h + 1], axis=0))
                kp4f = spool.tile([128, HLOC], f32, tag="kp4f")
                nc.gpsimd.tensor_copy(kp4f[:], kp4[:])
                vo4 = spool.tile([128, HLOC], f32, tag="vo4")
                nc.gpsimd.tensor_add(vo4[:], kp4f[:], hb_t[:])
                vo4i = spool.tile([128, HLOC], i32, tag="vo4i")
                nc.gpsimd.tensor_copy(vo4i[:], vo4[:])
                y_t = ypool.tile([128, 256], f32, tag="y")
                if no_idma:
                    nc.vector.memset(y_t[:], 0.0)
                else:
                    for h in range(HLOC):
                        nc.gpsimd.indirect_dma_start(
                            out=y_t[:, h * DK:(h + 1) * DK], out_offset=None,
                            in_=vprime,
                            in_offset=bass.IndirectOffsetOnAxis(
                                ap=vo4i[:, h:h + 1], axis=0))
                if debug:
                    nc.sync.dma_start(d_ks[:, qt * HLOC:(qt + 1) * HLOC], ks4[:])
                    nc.sync.dma_start(d_kp[:, qt * HLOC:(qt + 1) * HLOC], kp4f[:])
                    if qt == 0:
                        nc.sync.dma_start(d_y, y_t[:])

                for cb, dst in ((0, yt_a), (1, yt_b)):
                    tp = tps.tile([128, 128], f32, space="PSUM", tag="tp")
                    nc.tensor.transpose(tp[:], y_t[:, cb * 128:(cb + 1) * 128],
                                        ident[:])
                    nc.scalar.copy(dst[:, qt * 128:(qt + 1) * 128], tp[:])

            if debug:
                nc.sync.dma_start(d_yta, yt_a[:].bitcast(f32))
            nc.sync.dma_start(wo_t[:], woT)
            nc.vector.tensor_copy(wo_r[:], wo_t[:])
            for st in range(QT if not no_wo else 0):
                for ob in range(2):
                    ps = wps.tile([128, QUARTER], f32, space="PSUM", tag="wops")
                    nc.tensor.matmul(
                        ps[:], yt_a[:, st * 128:(st + 1) * 128],
                        wo_r[:, 0, ob * QUARTER:(ob + 1) * QUARTER],
                        start=True, stop=False)
                    nc.tensor.matmul(
                        ps[:], yt_b[:, st * 128:(st + 1) * 128],
                        wo_r[:, 1, ob * QUARTER:(ob + 1) * QUARTER],
                        start=False, stop=True)
                    o_sb = opool.tile([128, QUARTER], f32, tag="osb")
                    nc.scalar.copy(o_sb[:], ps[:])
                    nc.sync.dma_start(
                        out_part[st * 128:(st + 1) * 128,
                                 ob * QUARTER:(ob + 1) * QUARTER],
                        o_sb[:])

    nc.compile()
    return nc


def _get_program(nq, has_bq, has_bk, has_bv):
    key = (tuple(nq), has_bq, has_bk, has_bv)
    if key not in _prog_cache:
        _prog_cache[key] = _build_program(nq, has_bq, has_bk, has_bv)
    return _prog_cache[key]


def _gumbel_keep():
    """Reproduce the reference's PRNG streams on host CPU."""
    import jax
    cpu = jax.devices("cpu")[0]
    with jax.default_device(cpu):
        import jax.numpy as jnp
        nkey = jax.random.key(42)
        k_g, k_d = jax.random.split(nkey)
        shape = (B, H, S, S)
        u = jax.random.uniform(k_g, shape, dtype=jnp.float32)
        gumbel = -jnp.log(-jnp.log(u + EPS) + EPS)
        keep = jax.random.bernoulli(k_d, 1.0 - DROP_P, shape)
        return np.asarray(gumbel), np.asarray(keep)


_rand_cache = {}


def _core_inputs(x, W_q, W_k, W_v, W_o, b_q, b_k, b_v, gm, vidx):
    p_idx = np.arange(128, dtype=np.float32)
    iota = np.broadcast_to(np.arange(S, dtype=np.float32), (128, S)).copy()
    idn = np.eye(128, dtype=np.float32)
    qt_idx = np.arange(QT, dtype=np.float32)
    h_idx = np.arange(HLOC, dtype=np.float32)
    rbase = ((h_idx[None, None, :] * S + qt_idx[None, :, None] * 128
              + p_idx[:, None, None]) * S).astype(np.float32)
    hbase = np.broadcast_to(h_idx[None, :] * (S + 1),
                            (128, HLOC)).astype(np.float32).copy()

    in_maps = []
    for core in range(NCORES):
        b = core // 4
        hg = core % 4
        hsl = slice(hg * HLOC, (hg + 1) * HLOC)

        xT_re = np.ascontiguousarray(x[b].T.reshape(8, 128, S).transpose(1, 0, 2))

        def w_re(W, scale):
            W_loc = W[hg * 256:(hg + 1) * 256, :] * scale
            return np.ascontiguousarray(
                W_loc.T.reshape(8, 128, 256).transpose(1, 0, 2)).astype(np.float32)

        wq_re = w_re(W_q, 1.0 / math.sqrt(DK))
        wk_re = w_re(W_k, 1.0)
        wv_re = w_re(W_v, 1.0 / (1.0 - DROP_P))

        Wo_loc = W_o[:, hg * 256:(hg + 1) * 256]
        woT_re = np.ascontiguousarray(
            Wo_loc.T.reshape(2, 128, D).transpose(1, 0, 2)).astype(np.float32)

        gmg_c = np.ascontiguousarray(gm[b, hsl])
        vidx_c = vidx[b, hsl].reshape(-1, 1)
        vidxf = np.ascontiguousarray(np.concatenate(
            [vidx_c, np.zeros((4096, 1), np.uint16)], axis=0))

        bq_re = np.ascontiguousarray(
            (b_q[hg * 256:(hg + 1) * 256] / math.sqrt(DK)).reshape(2, 128).T
        ).astype(np.float32)
        bk_re = np.ascontiguousarray(
            b_k[hg * 256:(hg + 1) * 256].reshape(2, 128).T).astype(np.float32)
        bv_re = (b_v[hg * 256:(hg + 1) * 256] / (1.0 - DROP_P)
                 ).reshape(1, 256).astype(np.float32)

        in_maps.append(dict(
            xT=xT_re, wq=wq_re, wk=wk_re, wv=wv_re, woT=woT_re,
            gmg=gmg_c, iot=iota, rbase=rbase, hbase=hbase, vidxf=vidxf,
            bq2=bq_re, bk2=bk_re, bvrow=bv_re, idn=idn,
        ))
    return in_maps


def kernel(**inputs):
    import concourse.bass_utils as bass_utils

    x = np.asarray(inputs["x"], dtype=np.float32)
    W_q = np.asarray(inputs["W_q"], dtype=np.float32)
    W_k = np.asarray(inputs["W_k"], dtype=np.float32)
    W_v = np.asarray(inputs["W_v"], dtype=np.float32)
    W_o = np.asarray(inputs["W_o"], dtype=np.float32)
    b_q = np.asarray(inputs["b_q"], dtype=np.float32)
    b_k = np.asarray(inputs["b_k"], dtype=np.float32)
    b_v = np.asarray(inputs["b_v"], dtype=np.float32)
    b_o = np.asarray(inputs["b_o"], dtype=np.float32)
    mask = np.asarray(inputs["mask"])
    hard = int(np.asarray(inputs["hard"]))

    if hard != 1:
        raise NotImplementedError("only hard=1 supported")

    if "gk" not in _rand_cache:
        _rand_cache["gk"] = _gumbel_keep()
    gumbel, keep = _rand_cache["gk"]

    gm = np.where((mask[:, None, :, :] == 0), np.float32(-np.inf), gumbel)
    ar = np.arange(S, dtype=np.uint16)
    vidx = np.where(keep, ar[None, None, None, :], np.uint16(S))

    nq = []
    anymask = (mask != 0)
    for qt in range(QT):
        sub = anymask[:, qt * 128:(qt + 1) * 128, :]
        cols = np.nonzero(sub.any(axis=(0, 1)))[0]
        lastk = int(cols.max()) if cols.size else 0
        nq.append(min(NQMAX, max(1, -(-(lastk + 1) // QUARTER))))
    nq = tuple(nq)

    has_bq = bool(np.any(b_q != 0))
    has_bk = bool(np.any(b_k != 0))
    has_bv = bool(np.any(b_v != 0))

    nc = _get_program(nq, has_bq, has_bk, has_bv)
    in_maps = _core_inputs(x, W_q, W_k, W_v, W_o, b_q, b_k, b_v, gm, vidx)
    global _last_in_maps
    _last_in_maps = in_maps

    res = bass_utils.run_bass_kernel_spmd(
        nc, in_maps, core_ids=list(range(NCORES)))

    out = np.zeros((B, S, D), dtype=np.float32)
    for core in range(NCORES):
        out[core // 4] += res.results[core]["out_part"]
    out += b_o[None, None, :]
    return out
